# revision 33
# baseline (speedup 1.0000x reference)
"""Trainium2 Bass kernel v2 for nn_AllObsPredictAtten (moe_routing).

Host-transposed staging: x is cast to bf16 and laid out feature-major
(partition-major per 128-row chunk) on the host, so the device program
has ZERO transposes and ZERO PSUM->SBUF evictions of activations --
it is a pure stream of matmuls over a ~22.5MB/core bf16 DMA load.

Per core (1024 batch rows, processed as 2 halves of N=512):
  - grid cells:  XG0/XG1 [128, 25, 1024] bf16, XG2 [44, 25, 1024]
  - inventory:   XI0/XI1 [128, 10, 1024] bf16, XI2 [44, 10, 1024]
  - onehot:      XOHA [112, 1024], XOHB [63, 1024] bf16
  - goal:        XGO0/XGO1 [128, 1024], XGO2 [44, 1024] bf16
  - hiddens:     HT [32, 1024] f32r
  - output:      OUT_T [32, 1024] f32, transposed back on host.

All weight transforms are precomputed on host (<1 MB, replicated).
softmax normalization is folded into the selection weights before the
output-layer matmuls.
"""
import sys

sys.path.insert(0, "/opt/trn_rl_repo")

import numpy as np
import ml_dtypes

import concourse.bacc as bacc
import concourse.bass as bass
import concourse.tile as tile
from concourse import mybir, bass2jax

F32 = mybir.dt.float32
F32R = mybir.dt.float32r
BF16 = mybir.dt.bfloat16

P = 128
BL = 1024           # batch rows per core
NCORES = 8
HID = 32

GRID0 = 0
OH0 = 7500
GOAL0 = 7675
INV0 = 7975
XW = 10975

_CACHE = {}


# ----------------------------------------------------------------------------
# host-side input + parameter staging
# ----------------------------------------------------------------------------

def _prep_params(i):
    bf = ml_dtypes.bfloat16
    f32 = np.float32
    p = {}

    def chunkT(W, dt):  # W [32, F] -> [128, nk, 32] transposed chunks
        F = W.shape[1]
        nk = (F + 127) // 128
        out = np.zeros((P, nk, 32), dtype=dt)
        for k in range(nk):
            sz = min(128, F - 128 * k)
            out[:sz, k, :] = W[:, 128 * k:128 * k + sz].T.astype(dt)
        return out

    p["wg"] = chunkT(i["W_embed"], bf)        # [128, 3, 32] bf16
    p["winv1"] = chunkT(i["W_inv1"], bf)
    p["wgoal"] = chunkT(i["W_goal"], bf)
    p["wcomb"] = chunkT(i["W_comb"], bf)      # [128, 7, 32] bf16
    p["winv2"] = chunkT(i["W_inv2"], bf)      # [128, 3, 32]

    # block-diag onehot weights: group g covers cells 4g..4g+3 (g6 = cell 24)
    woh = np.zeros((P, 7, P), dtype=bf)
    WohT = i["W_onehot"].T  # [7, 32]
    for g in range(7):
        cells = range(4 * g, min(4 * g + 4, 25))
        for ci, c in enumerate(cells):
            r = 7 * c if c < 16 else 7 * c - 112
            woh[r:r + 7, g, 32 * ci:32 * ci + 32] = WohT.astype(bf)
    p["woh_bd"] = woh

    # in-layer grouped: [32i, 3j, 128(4a x 32o)]
    win = np.zeros((32, 3, P), dtype=bf)
    for j in range(3):
        for a in range(4):
            win[:, j, 32 * a:32 * a + 32] = i["in_W"][4 * j + a].T.astype(bf)
    p["w_in_grp"] = win

    # out-layer stacked big-K: [128, 3, 32]: rows 32a+i = out_W[4j+a, o, i]
    wout = np.zeros((P, 3, 32), dtype=f32)
    for j in range(3):
        for a in range(4):
            wout[32 * a:32 * a + 32, j, :] = i["out_W"][4 * j + a].T
    p["w_out_big"] = wout

    def tailpair(W):
        t = np.zeros((P, 64), dtype=bf)
        t[0:44, 0:32] = W[:, 256:300].T.astype(bf)
        t[44:88, 32:64] = W[:, 256:300].T.astype(bf)
        return t

    p["w_tail_g"] = tailpair(i["W_embed"])
    p["w_tail_i"] = tailpair(i["W_inv1"])

    p["w_att"] = i["att_W"].T.astype(f32).copy()      # [32, 12]
    E3 = np.zeros((12, 3, P), dtype=f32)
    for j in range(3):
        for a in range(4):
            E3[4 * j + a, j, 32 * a:32 * a + 32] = 1.0
    p["E3"] = E3
    p["ones12"] = np.ones((12, 1), dtype=f32)
    p["ones112"] = np.ones((1, 12), dtype=f32)
    p["outb"] = i["out_b"].astype(f32).copy()          # [12, 32]

    p["b_c1x4"] = np.tile(i["b_embed"], 4)[:, None].astype(f32)
    p["b_ohx4"] = np.tile(i["b_onehot"], 4)[:, None].astype(f32)
    p["b_i1x4"] = np.tile(i["b_inv1"], 4)[:, None].astype(f32)
    p["b_comb"] = i["b_comb"][:, None].astype(f32).copy()
    p["b_inv2"] = i["b_inv2"][:, None].astype(f32).copy()
    p["b_goal"] = i["b_goal"][:, None].astype(f32).copy()
    p["in_b_stk"] = i["in_b"].reshape(3, 128).T.astype(f32).copy()  # [128, 3]
    p["att_b"] = i["att_b"][:, None].astype(f32).copy()
    return p


def _prep_x(x, hiddens):
    """x [8192, 10975] f32, hiddens [8192, 32] -> dict of sharded arrays.

    Each array's axis 0 is (core * per_core_partition) for PartitionSpec
    sharding; layout is partition-major so every DMA descriptor is a
    contiguous >=2KB run on both the DRAM and SBUF side.
    """
    bf = ml_dtypes.bfloat16
    xb = np.asarray(x, dtype=np.float32).reshape(NCORES, BL, XW)
    d = {}

    def pack(region, ncells, groups):
        # region [c, B, ncells, 300] -> slots [c, 128, nslot, B]
        # per 4-cell group: 8 k-slots (cell-major) + 2 tail-pair slots;
        # 2-cell group: 4 k-slots + 1 pair; 1-cell group: 2 k-slots + 1 tail
        nslot = sum({4: 10, 2: 5, 1: 3}[n] for n in groups)
        outp = np.zeros((NCORES, P, nslot, BL), dtype=bf)
        base = 0
        cell = 0
        for n in groups:
            for ci in range(n):
                cT = region[:, :, cell + ci, :].transpose(0, 2, 1)  # [c,300,B]
                outp[:, :, base + 2 * ci, :] = cT[:, 0:128].astype(bf)
                outp[:, :, base + 2 * ci + 1, :] = cT[:, 128:256].astype(bf)
            if n == 4:
                for pi in range(2):
                    cA = region[:, :, cell + 2 * pi, 256:300].transpose(0, 2, 1)
                    cB = region[:, :, cell + 2 * pi + 1, 256:300].transpose(0, 2, 1)
                    outp[:, 0:44, base + 8 + pi, :] = cA.astype(bf)
                    outp[:, 44:88, base + 8 + pi, :] = cB.astype(bf)
                base += 10
            elif n == 2:
                cA = region[:, :, cell, 256:300].transpose(0, 2, 1)
                cB = region[:, :, cell + 1, 256:300].transpose(0, 2, 1)
                outp[:, 0:44, base + 4, :] = cA.astype(bf)
                outp[:, 44:88, base + 4, :] = cB.astype(bf)
                base += 5
            else:
                cA = region[:, :, cell, 256:300].transpose(0, 2, 1)
                outp[:, 0:44, base + 2, :] = cA.astype(bf)
                base += 3
            cell += n
        return outp.reshape(-1, nslot, BL)

    grid = xb[:, :, GRID0:OH0].reshape(NCORES, BL, 25, 300)
    d["xgp"] = pack(grid, 25, [4, 4, 4, 4, 4, 4, 1])
    inv = xb[:, :, INV0:].reshape(NCORES, BL, 10, 300)
    d["xip"] = pack(inv, 10, [4, 4, 2])
    d["xoha"] = np.ascontiguousarray(
        xb[:, :, OH0:OH0 + 112].transpose(0, 2, 1)).astype(bf).reshape(-1, BL)
    d["xohb"] = np.ascontiguousarray(
        xb[:, :, OH0 + 112:OH0 + 175].transpose(0, 2, 1)).astype(bf).reshape(-1, BL)
    d["xgo0"] = np.ascontiguousarray(
        xb[:, :, GOAL0:GOAL0 + 128].transpose(0, 2, 1)).astype(bf).reshape(-1, BL)
    d["xgo1"] = np.ascontiguousarray(
        xb[:, :, GOAL0 + 128:GOAL0 + 256].transpose(0, 2, 1)).astype(bf).reshape(-1, BL)
    d["xgo2"] = np.ascontiguousarray(
        xb[:, :, GOAL0 + 256:GOAL0 + 300].transpose(0, 2, 1)).astype(bf).reshape(-1, BL)
    h = np.asarray(hiddens, dtype=np.float32).reshape(NCORES, BL, HID)
    d["ht"] = np.ascontiguousarray(h.transpose(0, 2, 1)).reshape(-1, BL)
    return d


_X_DECLS = {
    "xgp": ([P, 63, BL], BF16),
    "xip": ([P, 25, BL], BF16),
    "xoha": ([112, BL], BF16),
    "xohb": ([63, BL], BF16),
    "xgo0": ([P, BL], BF16),
    "xgo1": ([P, BL], BF16),
    "xgo2": ([44, BL], BF16),
    "ht": ([HID, BL], F32R),
}

_PARAM_DECLS = {
    "wg": ([P, 3, 32], BF16),
    "winv1": ([P, 3, 32], BF16),
    "wgoal": ([P, 3, 32], BF16),
    "wcomb": ([P, 7, 32], BF16),
    "winv2": ([P, 3, 32], BF16),
    "woh_bd": ([P, 7, P], BF16),
    "w_tail_g": ([P, 64], BF16),
    "w_tail_i": ([P, 64], BF16),
    "w_in_grp": ([32, 3, P], BF16),
    "w_out_big": ([P, 3, 32], F32R),
    "w_att": ([32, 12], F32R),
    "E3": ([12, 3, P], F32R),
    "ones12": ([12, 1], F32R),
    "ones112": ([1, 12], F32R),
    "outb": ([12, 32], F32R),
    "b_c1x4": ([P, 1], F32),
    "b_ohx4": ([P, 1], F32),
    "b_i1x4": ([P, 1], F32),
    "b_comb": ([32, 1], F32),
    "b_inv2": ([32, 1], F32),
    "b_goal": ([32, 1], F32),
    "in_b_stk": ([P, 3], F32),
    "att_b": ([12, 1], F32),
}


# ----------------------------------------------------------------------------
# device program
# ----------------------------------------------------------------------------

def _build_nc(bench_r=0, unroll=1):
    import os
    IO_BUFS = int(os.environ.get("IO_BUFS2", "2"))
    C1_BUFS = int(os.environ.get("C1_BUFS2", "2"))
    nc = bacc.Bacc(None, target_bir_lowering=False)
    xin = {}
    for name, (shape, dt) in _X_DECLS.items():
        xin[name] = nc.declare_dram_parameter(name, shape, dt, isOutput=False)
    prm = {}
    for name, (shape, dt) in _PARAM_DECLS.items():
        prm[name] = nc.declare_dram_parameter(name, shape, dt, isOutput=False)
    OUT = nc.declare_dram_parameter("out_t", [HID, BL], F32, isOutput=True)

    with tile.TileContext(nc) as tc:
        with (
            tc.tile_pool(name="const", bufs=1) as cp,
            tc.tile_pool(name="io", bufs=IO_BUFS) as io,
            tc.tile_pool(name="res", bufs=1) as res,
            tc.tile_pool(name="sm", bufs=int(os.environ.get("SM_BUFS", "2"))) as sm,
            tc.tile_pool(name="wk", bufs=int(os.environ.get("WK_BUFS2", "2"))) as wk,
            tc.tile_pool(name="psC", bufs=C1_BUFS, space="PSUM") as psC,
            tc.tile_pool(name="psB", bufs=int(os.environ.get("C2_BUFS2", "1")),
                         space="PSUM") as psB,
            tc.tile_pool(name="ps", bufs=1, space="PSUM") as ps,
        ):
            # ---- constants to SBUF
            c = {}
            for name, (shape, dt) in _PARAM_DECLS.items():
                t = cp.tile(shape, dt, tag=name)
                nc.sync.dma_start(out=t[:], in_=prm[name].ap())
                c[name] = t

            import contextlib
            loop_ctx = tc.For_i(0, bench_r, 1) if bench_r > 1 else contextlib.nullcontext()

            with loop_ctx:
              for _rep in range(unroll):
                # ---- small inputs: onehot, goal, hiddens
                  toha = sm.tile([112, BL], BF16, tag="toha")
                  nc.sync.dma_start(out=toha[:], in_=xin["xoha"].ap())
                  tohb = sm.tile([63, BL], BF16, tag="tohb")
                  nc.sync.dma_start(out=tohb[:], in_=xin["xohb"].ap())
                  tgo = []
                  for k, nm in enumerate(("xgo0", "xgo1", "xgo2")):
                      t = sm.tile(list(_X_DECLS[nm][0]), BF16, tag=nm)
                      nc.sync.dma_start(out=t[:], in_=xin[nm].ap())
                      tgo.append(t)
                  tht = sm.tile([HID, BL], F32R, tag="tht")
                  nc.sync.dma_start(out=tht[:], in_=xin["ht"].ap())

                  HALF = (slice(0, 512), slice(512, 1024))

                  # ---- goal embedding + attention/selection, emitted as a
                  # list of deferred steps so each PE op in the chain hides
                  # under a later grid MM burst (PE is in-order: a dependent
                  # op placed early stalls every MM behind it).
                  goal_emb = [None, None]
                  sel_n = [None, None]
                  att_state = {}

                  def att_step_goal(s):
                      cols = HALF[s]
                      gps = ps.tile([32, 512], F32, tag="misc")
                      for k, Kk in enumerate((128, 128, 44)):
                          nc.tensor.matmul(
                              gps[:], c["wgoal"][0:Kk, k, :], tgo[k][0:Kk, cols],
                              start=(k == 0), stop=(k == 2),
                          )
                      ge = wk.tile([32, 512], BF16, tag="goal_emb")
                      nc.scalar.activation(ge[:], gps[:],
                                           mybir.ActivationFunctionType.Relu,
                                           bias=c["b_goal"][:])
                      goal_emb[s] = ge

                  def att_step_logits(s):
                      cols = HALF[s]
                      aps = ps.tile([12, 512], F32, tag="misc")
                      nc.tensor.matmul(aps[:], c["w_att"][:], tht[:, cols],
                                       start=True, stop=True)
                      expsel_f = wk.tile([12, 512], F32, tag="expsel_f")
                      nc.scalar.activation(expsel_f[:], aps[:],
                                           mybir.ActivationFunctionType.Exp,
                                           bias=c["att_b"][:])
                      expsel_r = wk.tile([12, 512], F32R, tag="expsel_r")
                      nc.vector.tensor_copy(expsel_r[:], expsel_f[:])
                      att_state[("exp", s)] = expsel_f
                      att_state[("expr", s)] = expsel_r

                  def att_step_sum(s):
                      sps = ps.tile([1, 512], F32, tag="misc")
                      nc.tensor.matmul(sps[:], c["ones12"][:],
                                       att_state[("expr", s)][:],
                                       start=True, stop=True)
                      recip = wk.tile([1, 512], F32, tag="recip")
                      nc.vector.reciprocal(recip[:], sps[:])
                      recip_r = wk.tile([1, 512], F32R, tag="recip_r")
                      nc.vector.tensor_copy(recip_r[:], recip[:])
                      att_state[("recipr", s)] = recip_r

                  def att_step_norm(s):
                      rps = ps.tile([12, 512], F32, tag="misc")
                      nc.tensor.matmul(rps[:], c["ones112"][:],
                                       att_state[("recipr", s)][:],
                                       start=True, stop=True)
                      seln_f = wk.tile([12, 512], F32, tag="seln_f")
                      nc.vector.tensor_mul(seln_f[:], att_state[("exp", s)][:],
                                           rps[:])
                      sn = wk.tile([12, 512], F32R, tag="sel_n")
                      nc.vector.tensor_copy(sn[:], seln_f[:])
                      sel_n[s] = sn

                  att_q = []
                  for s in range(2):
                      att_q += [
                          lambda s=s: att_step_goal(s),
                          lambda s=s: att_step_logits(s),
                          lambda s=s: att_step_sum(s),
                          lambda s=s: att_step_norm(s),
                      ]
                  pend = []

                  # ---- accumulators: [:, 0:512] = half 0, [:, 512:1024] = half 1
                  acc_comb = ps.tile([32, BL], F32, tag="acc_comb")
                  acc_inv = ps.tile([32, BL], F32, tag="acc_inv")

                  # ---- grid groups: c1 (embed) + c2 (onehot) -> comb accum
                  # 2-group (20-slot, 5.2MB) coalesced loads on alternating
                  # queues; the last group's 3 slots ride a small tile
                  import os as _os
                  _ABL = int(_os.environ.get("ABLATE_DMA", "0"))
                  gtiles = {}
                  for g in range(7):
                      ncell = 4 if g < 6 else 1
                      if g < 6:
                          chunk = g // 2
                          if g % 2 == 0:
                              tg = io.tile([P, 20, BL], BF16, tag="gq")
                              eng = nc.gpsimd if chunk % 2 == 0 else nc.sync
                              ld = 2 if _ABL else 20
                              eng.dma_start(
                                  out=tg[:, 0:ld, :],
                                  in_=xin["xgp"].ap()[:, 20 * chunk:20 * chunk + ld, :])
                              gtiles[chunk] = tg
                          tq = gtiles[chunk]
                          base = (g % 2) * 10
                      else:
                          tq = io.tile([P, 5, BL], BF16, tag="gq5")
                          nc.sync.dma_start(out=tq[:, 0:3, :],
                                            in_=xin["xgp"].ap()[:, 60:63, :])
                          base = 0
                      Kq = 128 if g < 6 else 32
                      Mq = 32 * ncell
                      for s in range(2):
                          cols = HALF[s]
                          # front-load PE work that does not depend on the big
                          # grid DMA: pending comb MM, one attention step, c2
                          while pend:
                              pend.pop(0)()
                          if att_q:
                              att_q.pop(0)()
                          c2ps = psB.tile([P, 512], F32, tag="c2")
                          if g <= 3:
                              nc.tensor.matmul(c2ps[0:Mq, :],
                                               c["woh_bd"][0:112, g, 0:Mq],
                                               toha[:, cols], start=True, stop=True)
                          else:
                              nc.tensor.matmul(c2ps[0:Mq, :],
                                               c["woh_bd"][0:63, g, 0:Mq],
                                               tohb[:, cols], start=True, stop=True)
                          c1ps = psC.tile([P, 512], F32, tag="c1")
                          for ci in range(ncell):
                              for k in range(2):
                                  nc.tensor.matmul(
                                      c1ps[32 * ci:32 * ci + 32, :],
                                      c["wg"][:, k, :],
                                      tq[:, base + 2 * ci + k, cols],
                                      start=(k == 0), stop=False,
                                      tile_position=(0, 32 * ci),
                                  )
                          if g < 6:
                              for pi in range(2):
                                  nc.tensor.matmul(
                                      c1ps[64 * pi:64 * pi + 64, :],
                                      c["w_tail_g"][0:88, :],
                                      tq[0:88, base + 8 + pi, cols],
                                      start=False, stop=True,
                                      tile_position=(0, 64 * pi),
                                  )
                          else:
                              nc.tensor.matmul(
                                  c1ps[0:32, :],
                                  c["wg"][0:44, 2, :],
                                  tq[0:44, base + 2, cols],
                                  start=False, stop=True,
                                  tile_position=(0, 0),
                              )
                          c1r = wk.tile([P, 512], BF16, tag="c1r")
                          c2r = wk.tile([P, 512], BF16, tag="c2r")
                          nc.scalar.activation(c1r[0:Mq, :], c1ps[0:Mq, :],
                                               mybir.ActivationFunctionType.Relu,
                                               bias=c["b_c1x4"][0:Mq, :])
                          nc.scalar.activation(c2r[0:Mq, :], c2ps[0:Mq, :],
                                               mybir.ActivationFunctionType.Relu,
                                               bias=c["b_ohx4"][0:Mq, :])
                          c12 = wk.tile([P, 512], BF16, tag="c12")
                          nc.vector.tensor_add(c12[0:Mq, :], c1r[0:Mq, :],
                                               c2r[0:Mq, :])

                          def comb_mm(g=g, s=s, c12=c12, Kq=Kq):
                              nc.tensor.matmul(
                                  acc_comb[:, HALF[s]],
                                  c["wcomb"][0:Kq, g, :], c12[0:Kq, :],
                                  start=(g == 0), stop=(g == 6),
                              )
                          pend.append(comb_mm)

                  # ---- inventory groups -> inv2 accum
                  # one 20-slot load (iq0+iq1) + one 5-slot load (iq2)
                  ti = None
                  for iq in range(3):
                      ncell = (4, 4, 2)[iq]
                      if iq == 0:
                          ti = io.tile([P, 20, BL], BF16, tag="gq")
                          ld = 2 if _ABL else 20
                          nc.gpsimd.dma_start(out=ti[:, 0:ld, :],
                                              in_=xin["xip"].ap()[:, 0:ld, :])
                      if iq < 2:
                          tq = ti
                          base = iq * 10
                      else:
                          tq = io.tile([P, 5, BL], BF16, tag="gq5")
                          ld = 2 if _ABL else 5
                          nc.sync.dma_start(out=tq[:, 0:ld, :],
                                            in_=xin["xip"].ap()[:, 20:20 + ld, :])
                          base = 0
                      Kiq = (128, 128, 64)[iq]
                      Mq = 32 * ncell
                      for s in range(2):
                          cols = HALF[s]
                          # flush pending comb/acc MM under this burst
                          while pend:
                              pend.pop(0)()
                          if att_q:
                              att_q.pop(0)()
                          i1ps = psC.tile([P, 512], F32, tag="c1")
                          for ci in range(ncell):
                              for k in range(2):
                                  nc.tensor.matmul(
                                      i1ps[32 * ci:32 * ci + 32, :],
                                      c["winv1"][:, k, :],
                                      tq[:, base + 2 * ci + k, cols],
                                      start=(k == 0), stop=False,
                                      tile_position=(0, 32 * ci),
                                  )
                          for pi in range(ncell // 2):
                              nc.tensor.matmul(
                                  i1ps[64 * pi:64 * pi + 64, :],
                                  c["w_tail_i"][0:88, :],
                                  tq[0:88, base + 2 * ncell + pi, cols],
                                  start=False, stop=True,
                                  tile_position=(0, 64 * pi),
                              )
                          invr = wk.tile([P, 512], BF16, tag="invr")
                          nc.scalar.activation(invr[0:Mq, :], i1ps[0:Mq, :],
                                               mybir.ActivationFunctionType.Relu,
                                               bias=c["b_i1x4"][0:Mq, :])

                          def acc_mm(iq=iq, s=s, invr=invr, Kiq=Kiq):
                              nc.tensor.matmul(
                                  acc_inv[:, HALF[s]],
                                  c["winv2"][0:Kiq, iq, :], invr[0:Kiq, :],
                                  start=(iq == 0), stop=(iq == 2),
                              )
                          pend.append(acc_mm)

                  # ---- net embeddings + modules + weighted output per half
                  # (remaining deferred acc MMs are flushed interleaved so PE
                  # never sits on a raw ACT->MM dependency)
                  for s in range(2):
                      cols = HALF[s]
                      while pend:
                          pend.pop(0)()
                      grid_comb = wk.tile([32, 512], BF16, tag="grid_comb")
                      nc.scalar.activation(grid_comb[:], acc_comb[:, cols],
                                           mybir.ActivationFunctionType.Relu,
                                           bias=c["b_comb"][:])
                      inv_emb = wk.tile([32, 512], BF16, tag="inv_emb")
                      nc.scalar.activation(inv_emb[:], acc_inv[:, cols],
                                           mybir.ActivationFunctionType.Relu,
                                           bias=c["b_inv2"][:])

                      outps = ps.tile([32, 512], F32, tag="misc")
                      srcs = (grid_comb, inv_emb, goal_emb[s])
                      hps_l, Bps_l, hj_l, gr_l = [], [], [], []
                      # phase 1: all in-layer + E3 broadcast MMs (independent)
                      for j in range(3):
                          hps = psC.tile([P, 512], F32, tag="c1")
                          nc.tensor.matmul(hps[:], c["w_in_grp"][:, j, :],
                                           srcs[j][:], start=True, stop=True)
                          hps_l.append(hps)
                          hj = wk.tile([P, 512], F32, tag="hj")
                          nc.scalar.activation(hj[:], hps[:],
                                               mybir.ActivationFunctionType.Tanh,
                                               bias=c["in_b_stk"][:, j:j + 1])
                          hj_l.append(hj)
                          Bps = psB.tile([P, 512], F32, tag="c2")
                          nc.tensor.matmul(Bps[:], c["E3"][:, j, :], sel_n[s][:],
                                           start=True, stop=True)
                          Bps_l.append(Bps)
                          gr = wk.tile([P, 512], F32R, tag="gr")
                          nc.vector.tensor_mul(gr[:], hj[:], Bps[:])
                          gr_l.append(gr)
                      # phase 2: bias first (no deps), then weighted outputs
                      nc.tensor.matmul(outps[:], c["outb"][:], sel_n[s][:],
                                       start=True, stop=False)
                      for j in range(3):
                          nc.tensor.matmul(
                              outps[:], c["w_out_big"][:, j, :], gr_l[j][:],
                              start=False, stop=(j == 2),
                          )

                      out_sb = wk.tile([32, 512], F32, tag="out_sb")
                      nc.scalar.copy(out_sb[:], outps[:])
                      nc.scalar.dma_start(out=OUT.ap()[:, cols], in_=out_sb[:])

    nc.finalize()
    return nc


# ----------------------------------------------------------------------------
# 8-core runner (jit once, reuse)
# ----------------------------------------------------------------------------

def _make_runner(nc):
    import jax
    from jax.sharding import Mesh, PartitionSpec
    from jax.experimental.shard_map import shard_map

    bass2jax.install_neuronx_cc_hook()
    partition_name = nc.partition_id_tensor.name if nc.partition_id_tensor else None
    in_names, out_names, out_avals = [], [], []
    for alloc in nc.m.functions[0].allocations:
        if not isinstance(alloc, mybir.MemoryLocationSet):
            continue
        name = alloc.memorylocations[0].name
        if alloc.kind == "ExternalInput":
            if name != partition_name:
                in_names.append(name)
        elif alloc.kind == "ExternalOutput":
            out_names.append(name)
            out_avals.append(jax.core.ShapedArray(
                tuple(alloc.tensor_shape), mybir.dt.np(alloc.dtype)))
    n_params = len(in_names)
    n_outs = len(out_avals)
    in_names_full = in_names + out_names
    if partition_name is not None:
        in_names_full = in_names_full + [partition_name]
    donate = tuple(range(n_params, n_params + n_outs))

    def _body(*args):
        operands = list(args)
        if partition_name is not None:
            operands.append(bass2jax.partition_id_tensor())
        outs = bass2jax._bass_exec_p.bind(
            *operands,
            out_avals=tuple(out_avals),
            in_names=tuple(in_names_full),
            out_names=tuple(out_names),
            lowering_input_output_aliases=(),
            sim_require_finite=True,
            sim_require_nnan=True,
            nc=nc,
        )
        return tuple(outs)

    devices = jax.devices()[:NCORES]
    mesh = Mesh(np.asarray(devices), ("core",))
    in_specs = (PartitionSpec("core"),) * (n_params + n_outs)
    out_specs = (PartitionSpec("core"),) * n_outs
    sharded = jax.jit(
        shard_map(_body, mesh=mesh, in_specs=in_specs, out_specs=out_specs,
                  check_rep=False),
        donate_argnums=donate, keep_unused=True,
    )

    _CACHE["sharded"] = sharded
    _CACHE["body"] = _body
    _CACHE["mesh"] = mesh
    _CACHE["in_names"] = in_names
    _CACHE["out_names"] = out_names
    _CACHE["out_avals"] = out_avals
    _CACHE["n_params"] = n_params

    def run(global_ins):
        ins = [global_ins[name] for name in in_names]
        zeros = [np.zeros((NCORES * a.shape[0], *a.shape[1:]), a.dtype)
                 for a in out_avals]
        outs = sharded(*ins, *zeros)
        import jax as _j
        _j.block_until_ready(outs)
        return {name: np.asarray(outs[i]) for i, name in enumerate(out_names)}

    return run


def _get_runner():
    if "runner" not in _CACHE:
        nc = _build_nc()
        _CACHE["runner"] = _make_runner(nc)
    return _CACHE["runner"]


def _global_ins(inputs):
    prm = _prep_params(inputs)
    global_ins = _prep_x(inputs["x"], inputs["hiddens"])
    for name in _PARAM_DECLS:
        a = prm[name]
        global_ins[name] = np.concatenate([a] * NCORES, axis=0)
    return global_ins


def kernel(**inputs):
    run = _get_runner()
    outs = run(_global_ins(inputs))
    out_t = outs["out_t"]                      # [8*32, 1024] f32
    return np.ascontiguousarray(
        out_t.reshape(NCORES, HID, BL).transpose(0, 2, 1).reshape(NCORES * BL, HID))



# revision 37
# speedup vs baseline: 1.3827x; 1.3827x over previous
"""Trainium2 Bass kernel v3 for nn_AllObsPredictAtten (moe_routing).

Host-transposed staging: x is cast to bf16 and laid out feature-major
(partition-major per 128-row chunk) on the host, so the device program
has ZERO transposes and ZERO PSUM->SBUF evictions of activations --
it is a pure stream of matmuls over a ~22.5MB/core bf16 DMA load.

v3 over v2 (102.6us -> 90.3us): the kernel is PE-bound (DMA-ablation
measured ~90us with loads cut 10x), so the wins are all on the PE side:
  - deferred-op scheduling: the comb/inv2 accumulation matmuls (which
    RAW-depend on ACT relus) and the goal/attention chain are emitted
    lag-1 under the NEXT group's matmul burst, so the in-order PE never
    sits on an ACT/DVE dependency;
  - c1+c2 pre-added on the (idle) DVE, halving the comb matmuls;
  - bf16 intermediates (c1r/c2r/c12/invr/goal_emb/grid_comb/inv_emb)
    and bf16 wcomb/winv2/w_in_grp weights.
DTYPE RULE (hard-won): every DVE tensor op and every matmul must have
UNIFORM operand dtypes (all-bf16 or all-f32/f32r).  A bf16xf32-PSUM
tensor_mul or an f32->bf16 tensor_copy silently corrupts every OTHER
output column on hardware while passing CoreSim bit-exact.

Per core (1024 batch rows, processed as 2 halves of N=512):
  - grid cells:  XGP [128, 63, 1024] bf16 (10 slots per 4-cell group)
  - inventory:   XIP [128, 25, 1024] bf16
  - onehot:      XOHA [112, 1024], XOHB [63, 1024] bf16
  - goal:        XGO0/XGO1 [128, 1024], XGO2 [44, 1024] bf16
  - hiddens:     HT [32, 1024] f32r
  - output:      OUT_T [32, 1024] f32, transposed back on host.

All weight transforms are precomputed on host (<1 MB, replicated).
softmax normalization is folded into the selection weights before the
output-layer matmuls.
"""
import sys

sys.path.insert(0, "/opt/trn_rl_repo")

import numpy as np
import ml_dtypes

import concourse.bacc as bacc
import concourse.bass as bass
import concourse.tile as tile
from concourse import mybir, bass2jax

F32 = mybir.dt.float32
F32R = mybir.dt.float32r
BF16 = mybir.dt.bfloat16

P = 128
BL = 1024           # batch rows per core
NCORES = 8
HID = 32

GRID0 = 0
OH0 = 7500
GOAL0 = 7675
INV0 = 7975
XW = 10975

_CACHE = {}


# ----------------------------------------------------------------------------
# host-side input + parameter staging
# ----------------------------------------------------------------------------

def _prep_params(i):
    bf = ml_dtypes.bfloat16
    f32 = np.float32
    p = {}

    def chunkT(W, dt):  # W [32, F] -> [128, nk, 32] transposed chunks
        F = W.shape[1]
        nk = (F + 127) // 128
        out = np.zeros((P, nk, 32), dtype=dt)
        for k in range(nk):
            sz = min(128, F - 128 * k)
            out[:sz, k, :] = W[:, 128 * k:128 * k + sz].T.astype(dt)
        return out

    p["wg"] = chunkT(i["W_embed"], bf)        # [128, 3, 32] bf16
    p["winv1"] = chunkT(i["W_inv1"], bf)
    p["wgoal"] = chunkT(i["W_goal"], bf)
    p["wcomb"] = chunkT(i["W_comb"], bf)      # [128, 7, 32] bf16
    p["winv2"] = chunkT(i["W_inv2"], bf)      # [128, 3, 32]

    # block-diag onehot weights: group g covers cells 4g..4g+3 (g6 = cell 24)
    woh = np.zeros((P, 7, P), dtype=bf)
    WohT = i["W_onehot"].T  # [7, 32]
    for g in range(7):
        cells = range(4 * g, min(4 * g + 4, 25))
        for ci, c in enumerate(cells):
            r = 7 * c if c < 16 else 7 * c - 112
            woh[r:r + 7, g, 32 * ci:32 * ci + 32] = WohT.astype(bf)
    p["woh_bd"] = woh

    # in-layer grouped: [32i, 3j, 128(4a x 32o)]
    win = np.zeros((32, 3, P), dtype=bf)
    for j in range(3):
        for a in range(4):
            win[:, j, 32 * a:32 * a + 32] = i["in_W"][4 * j + a].T.astype(bf)
    p["w_in_grp"] = win

    # out-layer stacked big-K: [128, 3, 32]: rows 32a+i = out_W[4j+a, o, i]
    wout = np.zeros((P, 3, 32), dtype=f32)
    for j in range(3):
        for a in range(4):
            wout[32 * a:32 * a + 32, j, :] = i["out_W"][4 * j + a].T
    p["w_out_big"] = wout

    def tailpair(W):
        t = np.zeros((P, 64), dtype=bf)
        t[0:44, 0:32] = W[:, 256:300].T.astype(bf)
        t[44:88, 32:64] = W[:, 256:300].T.astype(bf)
        return t

    p["w_tail_g"] = tailpair(i["W_embed"])
    p["w_tail_i"] = tailpair(i["W_inv1"])

    p["w_att"] = i["att_W"].T.astype(f32).copy()      # [32, 12]
    E3 = np.zeros((12, 3, P), dtype=f32)
    for j in range(3):
        for a in range(4):
            E3[4 * j + a, j, 32 * a:32 * a + 32] = 1.0
    p["E3"] = E3
    p["ones12"] = np.ones((12, 1), dtype=f32)
    p["ones112"] = np.ones((1, 12), dtype=f32)
    p["outb"] = i["out_b"].astype(f32).copy()          # [12, 32]

    p["b_c1x4"] = np.tile(i["b_embed"], 4)[:, None].astype(f32)
    p["b_ohx4"] = np.tile(i["b_onehot"], 4)[:, None].astype(f32)
    p["b_i1x4"] = np.tile(i["b_inv1"], 4)[:, None].astype(f32)
    p["b_comb"] = i["b_comb"][:, None].astype(f32).copy()
    p["b_inv2"] = i["b_inv2"][:, None].astype(f32).copy()
    p["b_goal"] = i["b_goal"][:, None].astype(f32).copy()
    p["in_b_stk"] = i["in_b"].reshape(3, 128).T.astype(f32).copy()  # [128, 3]
    p["att_b"] = i["att_b"][:, None].astype(f32).copy()
    return p


def _prep_x(x, hiddens):
    """x [8192, 10975] f32, hiddens [8192, 32] -> dict of sharded arrays.

    Each array's axis 0 is (core * per_core_partition) for PartitionSpec
    sharding; layout is partition-major so every DMA descriptor is a
    contiguous >=2KB run on both the DRAM and SBUF side.
    """
    bf = ml_dtypes.bfloat16
    xb = np.asarray(x, dtype=np.float32).reshape(NCORES, BL, XW)
    d = {}

    def pack(region, ncells, groups):
        # region [c, B, ncells, 300] -> slots [c, 128, nslot, B]
        # per 4-cell group: 8 k-slots (cell-major) + 2 tail-pair slots;
        # 2-cell group: 4 k-slots + 1 pair; 1-cell group: 2 k-slots + 1 tail
        nslot = sum({4: 10, 2: 5, 1: 3}[n] for n in groups)
        outp = np.zeros((NCORES, P, nslot, BL), dtype=bf)
        base = 0
        cell = 0
        for n in groups:
            for ci in range(n):
                cT = region[:, :, cell + ci, :].transpose(0, 2, 1)  # [c,300,B]
                outp[:, :, base + 2 * ci, :] = cT[:, 0:128].astype(bf)
                outp[:, :, base + 2 * ci + 1, :] = cT[:, 128:256].astype(bf)
            if n == 4:
                for pi in range(2):
                    cA = region[:, :, cell + 2 * pi, 256:300].transpose(0, 2, 1)
                    cB = region[:, :, cell + 2 * pi + 1, 256:300].transpose(0, 2, 1)
                    outp[:, 0:44, base + 8 + pi, :] = cA.astype(bf)
                    outp[:, 44:88, base + 8 + pi, :] = cB.astype(bf)
                base += 10
            elif n == 2:
                cA = region[:, :, cell, 256:300].transpose(0, 2, 1)
                cB = region[:, :, cell + 1, 256:300].transpose(0, 2, 1)
                outp[:, 0:44, base + 4, :] = cA.astype(bf)
                outp[:, 44:88, base + 4, :] = cB.astype(bf)
                base += 5
            else:
                cA = region[:, :, cell, 256:300].transpose(0, 2, 1)
                outp[:, 0:44, base + 2, :] = cA.astype(bf)
                base += 3
            cell += n
        return outp.reshape(-1, nslot, BL)

    grid = xb[:, :, GRID0:OH0].reshape(NCORES, BL, 25, 300)
    d["xgp"] = pack(grid, 25, [4, 4, 4, 4, 4, 4, 1])
    inv = xb[:, :, INV0:].reshape(NCORES, BL, 10, 300)
    d["xip"] = pack(inv, 10, [4, 4, 2])
    d["xoha"] = np.ascontiguousarray(
        xb[:, :, OH0:OH0 + 112].transpose(0, 2, 1)).astype(bf).reshape(-1, BL)
    d["xohb"] = np.ascontiguousarray(
        xb[:, :, OH0 + 112:OH0 + 175].transpose(0, 2, 1)).astype(bf).reshape(-1, BL)
    d["xgo0"] = np.ascontiguousarray(
        xb[:, :, GOAL0:GOAL0 + 128].transpose(0, 2, 1)).astype(bf).reshape(-1, BL)
    d["xgo1"] = np.ascontiguousarray(
        xb[:, :, GOAL0 + 128:GOAL0 + 256].transpose(0, 2, 1)).astype(bf).reshape(-1, BL)
    d["xgo2"] = np.ascontiguousarray(
        xb[:, :, GOAL0 + 256:GOAL0 + 300].transpose(0, 2, 1)).astype(bf).reshape(-1, BL)
    h = np.asarray(hiddens, dtype=np.float32).reshape(NCORES, BL, HID)
    d["ht"] = np.ascontiguousarray(h.transpose(0, 2, 1)).reshape(-1, BL)
    return d


_X_DECLS = {
    "xgp": ([P, 63, BL], BF16),
    "xip": ([P, 25, BL], BF16),
    "xoha": ([112, BL], BF16),
    "xohb": ([63, BL], BF16),
    "xgo0": ([P, BL], BF16),
    "xgo1": ([P, BL], BF16),
    "xgo2": ([44, BL], BF16),
    "ht": ([HID, BL], F32R),
}

_PARAM_DECLS = {
    "wg": ([P, 3, 32], BF16),
    "winv1": ([P, 3, 32], BF16),
    "wgoal": ([P, 3, 32], BF16),
    "wcomb": ([P, 7, 32], BF16),
    "winv2": ([P, 3, 32], BF16),
    "woh_bd": ([P, 7, P], BF16),
    "w_tail_g": ([P, 64], BF16),
    "w_tail_i": ([P, 64], BF16),
    "w_in_grp": ([32, 3, P], BF16),
    "w_out_big": ([P, 3, 32], F32R),
    "w_att": ([32, 12], F32R),
    "E3": ([12, 3, P], F32R),
    "ones12": ([12, 1], F32R),
    "ones112": ([1, 12], F32R),
    "outb": ([12, 32], F32R),
    "b_c1x4": ([P, 1], F32),
    "b_ohx4": ([P, 1], F32),
    "b_i1x4": ([P, 1], F32),
    "b_comb": ([32, 1], F32),
    "b_inv2": ([32, 1], F32),
    "b_goal": ([32, 1], F32),
    "in_b_stk": ([P, 3], F32),
    "att_b": ([12, 1], F32),
}


# ----------------------------------------------------------------------------
# device program
# ----------------------------------------------------------------------------

def _build_nc(bench_r=0, unroll=1):
    import os
    IO_BUFS = int(os.environ.get("IO_BUFS2", "3"))
    C1_BUFS = int(os.environ.get("C1_BUFS2", "2"))
    nc = bacc.Bacc(None, target_bir_lowering=False)
    xin = {}
    for name, (shape, dt) in _X_DECLS.items():
        xin[name] = nc.declare_dram_parameter(name, shape, dt, isOutput=False)
    prm = {}
    for name, (shape, dt) in _PARAM_DECLS.items():
        prm[name] = nc.declare_dram_parameter(name, shape, dt, isOutput=False)
    OUT = nc.declare_dram_parameter("out_t", [HID, BL], F32, isOutput=True)

    with tile.TileContext(nc) as tc:
        with (
            tc.tile_pool(name="const", bufs=1) as cp,
            tc.tile_pool(name="io", bufs=IO_BUFS) as io,
            tc.tile_pool(name="res", bufs=1) as res,
            tc.tile_pool(name="sm", bufs=int(os.environ.get("SM_BUFS", "2"))) as sm,
            tc.tile_pool(name="wk", bufs=int(os.environ.get("WK_BUFS2", "2"))) as wk,
            tc.tile_pool(name="psC", bufs=C1_BUFS, space="PSUM") as psC,
            tc.tile_pool(name="psB", bufs=int(os.environ.get("C2_BUFS2", "1")),
                         space="PSUM") as psB,
            tc.tile_pool(name="ps", bufs=1, space="PSUM") as ps,
        ):
            # ---- constants to SBUF
            c = {}
            for name, (shape, dt) in _PARAM_DECLS.items():
                t = cp.tile(shape, dt, tag=name)
                nc.sync.dma_start(out=t[:], in_=prm[name].ap())
                c[name] = t

            import contextlib
            loop_ctx = tc.For_i(0, bench_r, 1) if bench_r > 1 else contextlib.nullcontext()

            with loop_ctx:
              for _rep in range(unroll):
                # ---- small inputs: onehot, goal, hiddens
                  toha = sm.tile([112, BL], BF16, tag="toha")
                  nc.sync.dma_start(out=toha[:], in_=xin["xoha"].ap())
                  tohb = sm.tile([63, BL], BF16, tag="tohb")
                  nc.sync.dma_start(out=tohb[:], in_=xin["xohb"].ap())
                  tgo = []
                  for k, nm in enumerate(("xgo0", "xgo1", "xgo2")):
                      t = sm.tile(list(_X_DECLS[nm][0]), BF16, tag=nm)
                      nc.sync.dma_start(out=t[:], in_=xin[nm].ap())
                      tgo.append(t)
                  tht = sm.tile([HID, BL], F32R, tag="tht")
                  nc.sync.dma_start(out=tht[:], in_=xin["ht"].ap())

                  HALF = (slice(0, 512), slice(512, 1024))

                  # ---- goal embedding + attention/selection, emitted as a
                  # list of deferred steps so each PE op in the chain hides
                  # under a later grid MM burst (PE is in-order: a dependent
                  # op placed early stalls every MM behind it).
                  goal_emb = [None, None]
                  sel_n = [None, None]
                  att_state = {}

                  def att_step_goal(s):
                      cols = HALF[s]
                      gps = ps.tile([32, 512], F32, tag="misc")
                      for k, Kk in enumerate((128, 128, 44)):
                          nc.tensor.matmul(
                              gps[:], c["wgoal"][0:Kk, k, :], tgo[k][0:Kk, cols],
                              start=(k == 0), stop=(k == 2),
                          )
                      ge = wk.tile([32, 512], BF16, tag="goal_emb")
                      nc.scalar.activation(ge[:], gps[:],
                                           mybir.ActivationFunctionType.Relu,
                                           bias=c["b_goal"][:])
                      goal_emb[s] = ge

                  def att_step_logits(s):
                      cols = HALF[s]
                      aps = ps.tile([12, 512], F32, tag="misc")
                      nc.tensor.matmul(aps[:], c["w_att"][:], tht[:, cols],
                                       start=True, stop=True)
                      expsel_f = wk.tile([12, 512], F32, tag="expsel_f")
                      nc.scalar.activation(expsel_f[:], aps[:],
                                           mybir.ActivationFunctionType.Exp,
                                           bias=c["att_b"][:])
                      expsel_r = wk.tile([12, 512], F32R, tag="expsel_r")
                      nc.vector.tensor_copy(expsel_r[:], expsel_f[:])
                      att_state[("exp", s)] = expsel_f
                      att_state[("expr", s)] = expsel_r

                  def att_step_sum(s):
                      sps = ps.tile([1, 512], F32, tag="misc")
                      nc.tensor.matmul(sps[:], c["ones12"][:],
                                       att_state[("expr", s)][:],
                                       start=True, stop=True)
                      recip = wk.tile([1, 512], F32, tag="recip")
                      nc.vector.reciprocal(recip[:], sps[:])
                      recip_r = wk.tile([1, 512], F32R, tag="recip_r")
                      nc.vector.tensor_copy(recip_r[:], recip[:])
                      att_state[("recipr", s)] = recip_r

                  def att_step_norm(s):
                      rps = ps.tile([12, 512], F32, tag="misc")
                      nc.tensor.matmul(rps[:], c["ones112"][:],
                                       att_state[("recipr", s)][:],
                                       start=True, stop=True)
                      seln_f = wk.tile([12, 512], F32, tag="seln_f")
                      nc.vector.tensor_mul(seln_f[:], att_state[("exp", s)][:],
                                           rps[:])
                      sn = wk.tile([12, 512], F32R, tag="sel_n")
                      nc.vector.tensor_copy(sn[:], seln_f[:])
                      sel_n[s] = sn

                  att_q = []
                  for s in range(2):
                      att_q += [
                          lambda s=s: att_step_goal(s),
                          lambda s=s: att_step_logits(s),
                          lambda s=s: att_step_sum(s),
                          lambda s=s: att_step_norm(s),
                      ]
                  pend = []

                  # ---- accumulators: [:, 0:512] = half 0, [:, 512:1024] = half 1
                  acc_comb = ps.tile([32, BL], F32, tag="acc_comb")
                  acc_inv = ps.tile([32, BL], F32, tag="acc_inv")

                  # ---- grid groups: c1 (embed) + c2 (onehot) -> comb accum
                  import os as _os
                  _ABL = int(_os.environ.get("ABLATE_DMA", "0"))
                  for g in range(7):
                      ncell = 4 if g < 6 else 1
                      nslot = 10 if g < 6 else 3
                      tq = io.tile([P, 10, BL], BF16, tag="g01")
                      eng = nc.gpsimd if g % 2 == 0 else nc.sync
                      ldslot = 1 if _ABL else nslot
                      eng.dma_start(
                          out=tq[:, 0:ldslot, :],
                          in_=xin["xgp"].ap()[:, 10 * g:10 * g + ldslot, :])
                      Kq = 128 if g < 6 else 32
                      Mq = 32 * ncell
                      for s in range(2):
                          cols = HALF[s]
                          c1ps = psC.tile([P, 512], F32, tag="c1")
                          for ci in range(ncell):
                              for k in range(2):
                                  nc.tensor.matmul(
                                      c1ps[32 * ci:32 * ci + 32, :],
                                      c["wg"][:, k, :],
                                      tq[:, 2 * ci + k, cols],
                                      start=(k == 0), stop=False,
                                      tile_position=(0, 32 * ci),
                                  )
                          if g < 6:
                              for pi in range(2):
                                  nc.tensor.matmul(
                                      c1ps[64 * pi:64 * pi + 64, :],
                                      c["w_tail_g"][0:88, :],
                                      tq[0:88, 8 + pi, cols],
                                      start=False, stop=True,
                                      tile_position=(0, 64 * pi),
                                  )
                          else:
                              nc.tensor.matmul(
                                  c1ps[0:32, :],
                                  c["wg"][0:44, 2, :],
                                  tq[0:44, 2, cols],
                                  start=False, stop=True,
                                  tile_position=(0, 0),
                              )
                          c2ps = psB.tile([P, 512], F32, tag="c2")
                          if g <= 3:
                              nc.tensor.matmul(c2ps[0:Mq, :],
                                               c["woh_bd"][0:112, g, 0:Mq],
                                               toha[:, cols], start=True, stop=True)
                          else:
                              nc.tensor.matmul(c2ps[0:Mq, :],
                                               c["woh_bd"][0:63, g, 0:Mq],
                                               tohb[:, cols], start=True, stop=True)
                          # flush deferred PE work under this burst's shadow:
                          # previous burst's comb MM (lag 1), then one
                          # attention-chain step
                          while pend:
                              pend.pop(0)()
                          if att_q:
                              att_q.pop(0)()
                          c1r = wk.tile([P, 512], BF16, tag="c1r")
                          c2r = wk.tile([P, 512], BF16, tag="c2r")
                          nc.scalar.activation(c1r[0:Mq, :], c1ps[0:Mq, :],
                                               mybir.ActivationFunctionType.Relu,
                                               bias=c["b_c1x4"][0:Mq, :])
                          nc.scalar.activation(c2r[0:Mq, :], c2ps[0:Mq, :],
                                               mybir.ActivationFunctionType.Relu,
                                               bias=c["b_ohx4"][0:Mq, :])
                          c12 = wk.tile([P, 512], BF16, tag="c12")
                          nc.vector.tensor_add(c12[0:Mq, :], c1r[0:Mq, :],
                                               c2r[0:Mq, :])

                          def comb_mm(g=g, s=s, c12=c12, Kq=Kq):
                              nc.tensor.matmul(
                                  acc_comb[:, HALF[s]],
                                  c["wcomb"][0:Kq, g, :], c12[0:Kq, :],
                                  start=(g == 0), stop=(g == 6),
                              )
                          pend.append(comb_mm)

                  # ---- inventory groups -> inv2 accum
                  ISLOT = (0, 10, 20)
                  for iq in range(3):
                      ncell = (4, 4, 2)[iq]
                      nslot = (10, 10, 5)[iq]
                      tq = io.tile([P, 10, BL], BF16, tag="g01")
                      eng = nc.gpsimd if iq % 2 == 0 else nc.sync
                      ldslot = 1 if _ABL else nslot
                      eng.dma_start(
                          out=tq[:, 0:ldslot, :],
                          in_=xin["xip"].ap()[:, ISLOT[iq]:ISLOT[iq] + ldslot, :])
                      Kiq = (128, 128, 64)[iq]
                      Mq = 32 * ncell
                      for s in range(2):
                          cols = HALF[s]
                          i1ps = psC.tile([P, 512], F32, tag="c1")
                          for ci in range(ncell):
                              for k in range(2):
                                  nc.tensor.matmul(
                                      i1ps[32 * ci:32 * ci + 32, :],
                                      c["winv1"][:, k, :],
                                      tq[:, 2 * ci + k, cols],
                                      start=(k == 0), stop=False,
                                      tile_position=(0, 32 * ci),
                                  )
                          for pi in range(ncell // 2):
                              nc.tensor.matmul(
                                  i1ps[64 * pi:64 * pi + 64, :],
                                  c["w_tail_i"][0:88, :],
                                  tq[0:88, 2 * ncell + pi, cols],
                                  start=False, stop=True,
                                  tile_position=(0, 64 * pi),
                              )
                          # flush pending comb/acc MM under this burst
                          while pend:
                              pend.pop(0)()
                          if att_q:
                              att_q.pop(0)()
                          invr = wk.tile([P, 512], BF16, tag="invr")
                          nc.scalar.activation(invr[0:Mq, :], i1ps[0:Mq, :],
                                               mybir.ActivationFunctionType.Relu,
                                               bias=c["b_i1x4"][0:Mq, :])

                          def acc_mm(iq=iq, s=s, invr=invr, Kiq=Kiq):
                              nc.tensor.matmul(
                                  acc_inv[:, HALF[s]],
                                  c["winv2"][0:Kiq, iq, :], invr[0:Kiq, :],
                                  start=(iq == 0), stop=(iq == 2),
                              )
                          pend.append(acc_mm)

                  # ---- net embeddings + modules + weighted output per half
                  # (remaining deferred acc MMs are flushed interleaved so PE
                  # never sits on a raw ACT->MM dependency)
                  for s in range(2):
                      cols = HALF[s]
                      while pend:
                          pend.pop(0)()
                      grid_comb = wk.tile([32, 512], BF16, tag="grid_comb")
                      nc.scalar.activation(grid_comb[:], acc_comb[:, cols],
                                           mybir.ActivationFunctionType.Relu,
                                           bias=c["b_comb"][:])
                      inv_emb = wk.tile([32, 512], BF16, tag="inv_emb")
                      nc.scalar.activation(inv_emb[:], acc_inv[:, cols],
                                           mybir.ActivationFunctionType.Relu,
                                           bias=c["b_inv2"][:])

                      outps = ps.tile([32, 512], F32, tag="misc")
                      srcs = (grid_comb, inv_emb, goal_emb[s])
                      hps_l, Bps_l, hj_l, gr_l = [], [], [], []
                      # phase 1: all in-layer + E3 broadcast MMs (independent)
                      for j in range(3):
                          hps = psC.tile([P, 512], F32, tag="c1")
                          nc.tensor.matmul(hps[:], c["w_in_grp"][:, j, :],
                                           srcs[j][:], start=True, stop=True)
                          hps_l.append(hps)
                          hj = wk.tile([P, 512], F32, tag="hj")
                          nc.scalar.activation(hj[:], hps[:],
                                               mybir.ActivationFunctionType.Tanh,
                                               bias=c["in_b_stk"][:, j:j + 1])
                          hj_l.append(hj)
                          Bps = psB.tile([P, 512], F32, tag="c2")
                          nc.tensor.matmul(Bps[:], c["E3"][:, j, :], sel_n[s][:],
                                           start=True, stop=True)
                          Bps_l.append(Bps)
                          gr = wk.tile([P, 512], F32R, tag="gr")
                          nc.vector.tensor_mul(gr[:], hj[:], Bps[:])
                          gr_l.append(gr)
                      # phase 2: bias first (no deps), then weighted outputs
                      nc.tensor.matmul(outps[:], c["outb"][:], sel_n[s][:],
                                       start=True, stop=False)
                      for j in range(3):
                          nc.tensor.matmul(
                              outps[:], c["w_out_big"][:, j, :], gr_l[j][:],
                              start=False, stop=(j == 2),
                          )

                      out_sb = wk.tile([32, 512], F32, tag="out_sb")
                      nc.scalar.copy(out_sb[:], outps[:])
                      nc.scalar.dma_start(out=OUT.ap()[:, cols], in_=out_sb[:])

    nc.finalize()
    return nc


# ----------------------------------------------------------------------------
# 8-core runner (jit once, reuse)
# ----------------------------------------------------------------------------

def _make_runner(nc):
    import jax
    from jax.sharding import Mesh, PartitionSpec
    from jax.experimental.shard_map import shard_map

    bass2jax.install_neuronx_cc_hook()
    partition_name = nc.partition_id_tensor.name if nc.partition_id_tensor else None
    in_names, out_names, out_avals = [], [], []
    for alloc in nc.m.functions[0].allocations:
        if not isinstance(alloc, mybir.MemoryLocationSet):
            continue
        name = alloc.memorylocations[0].name
        if alloc.kind == "ExternalInput":
            if name != partition_name:
                in_names.append(name)
        elif alloc.kind == "ExternalOutput":
            out_names.append(name)
            out_avals.append(jax.core.ShapedArray(
                tuple(alloc.tensor_shape), mybir.dt.np(alloc.dtype)))
    n_params = len(in_names)
    n_outs = len(out_avals)
    in_names_full = in_names + out_names
    if partition_name is not None:
        in_names_full = in_names_full + [partition_name]
    donate = tuple(range(n_params, n_params + n_outs))

    def _body(*args):
        operands = list(args)
        if partition_name is not None:
            operands.append(bass2jax.partition_id_tensor())
        outs = bass2jax._bass_exec_p.bind(
            *operands,
            out_avals=tuple(out_avals),
            in_names=tuple(in_names_full),
            out_names=tuple(out_names),
            lowering_input_output_aliases=(),
            sim_require_finite=True,
            sim_require_nnan=True,
            nc=nc,
        )
        return tuple(outs)

    devices = jax.devices()[:NCORES]
    mesh = Mesh(np.asarray(devices), ("core",))
    in_specs = (PartitionSpec("core"),) * (n_params + n_outs)
    out_specs = (PartitionSpec("core"),) * n_outs
    sharded = jax.jit(
        shard_map(_body, mesh=mesh, in_specs=in_specs, out_specs=out_specs,
                  check_rep=False),
        donate_argnums=donate, keep_unused=True,
    )

    _CACHE["sharded"] = sharded
    _CACHE["body"] = _body
    _CACHE["mesh"] = mesh
    _CACHE["in_names"] = in_names
    _CACHE["out_names"] = out_names
    _CACHE["out_avals"] = out_avals
    _CACHE["n_params"] = n_params

    def run(global_ins):
        ins = [global_ins[name] for name in in_names]
        zeros = [np.zeros((NCORES * a.shape[0], *a.shape[1:]), a.dtype)
                 for a in out_avals]
        outs = sharded(*ins, *zeros)
        import jax as _j
        _j.block_until_ready(outs)
        return {name: np.asarray(outs[i]) for i, name in enumerate(out_names)}

    return run


def _get_runner():
    if "runner" not in _CACHE:
        nc = _build_nc()
        _CACHE["runner"] = _make_runner(nc)
    return _CACHE["runner"]


def _global_ins(inputs):
    prm = _prep_params(inputs)
    global_ins = _prep_x(inputs["x"], inputs["hiddens"])
    for name in _PARAM_DECLS:
        a = prm[name]
        global_ins[name] = np.concatenate([a] * NCORES, axis=0)
    return global_ins


def kernel(**inputs):
    run = _get_runner()
    outs = run(_global_ins(inputs))
    out_t = outs["out_t"]                      # [8*32, 1024] f32
    return np.ascontiguousarray(
        out_t.reshape(NCORES, HID, BL).transpose(0, 2, 1).reshape(NCORES * BL, HID))

